# revision 1
# baseline (speedup 1.0000x reference)
"""ChebGCN (K=2, 2-layer) on 8 Trainium2 NeuronCores.

Full inputs in, full output out. Internally:
  - nodes partitioned by id across 8 cores (graph-parallel, per sharding hint)
  - per-core dest nodes bin-packed into 49 blocks x 128 slots (balanced)
  - messages reduced to post-weight space first: tx1@W1 == segsum(norm * (x@W1)[col])
  - gather tables in HBM, rows = 256B; dma_gather (int16 idx => lo/hi table halves)
  - scatter-add via one-hot matmuls accumulating in PSUM per dest block
  - layer-2 source features exchanged with an AllGather collective
Host does sharding prep (sort/pad/index building) and output reassembly only.
"""
import sys

for _p in ("/opt/trn_rl_repo",):
    if _p not in sys.path:
        sys.path.insert(0, _p)

import numpy as np
import concourse.bass as bass
import concourse.bacc as bacc
import concourse.mybir as mybir
import concourse.tile as tile
from concourse.bass_utils import run_bass_kernel_spmd

N = 50000
E = 800000
NCORE = 8
SH = 6250           # nodes per core
NB = 49             # dest blocks per core
P = 128
TPC = NB * P        # 6272 table rows per core
TR = NCORE * TPC    # 50176 table rows
HALF = 32768
F_IN, F_HID, F_OUT = 96, 64, 40
FP = 64             # padded feature dim (256B rows)
G = 16              # chunks per dma_gather group (overridden per attempt)

dt = mybir.dt


# ----------------------------------------------------------------- host prep
def _bin_pack_blocks(deg_local):
    order = np.argsort(-deg_local, kind="stable")
    loads = np.zeros(NB, np.int64)
    counts = np.zeros(NB, np.int32)
    slot = np.full(SH, -1, np.int64)
    big = np.iinfo(np.int64).max
    for l in order:
        b = int(np.argmin(np.where(counts < P, loads, big)))
        slot[l] = b * P + counts[b]
        counts[b] += 1
        loads[b] += deg_local[l]
    return slot


def _build_plan(edge_index):
    row = np.asarray(edge_index[0], np.int64)
    col = np.asarray(edge_index[1], np.int64)
    deg = np.bincount(row, minlength=N).astype(np.float32)
    dis = np.where(deg > 0, 1.0 / np.sqrt(np.maximum(deg, 1e-12)), 0.0).astype(np.float32)
    norm = (-dis[row] * dis[col]).astype(np.float32)

    slot_of_node = np.zeros(N, np.int64)
    pi_inv = np.full((NCORE, TPC), -1, np.int64)
    for c in range(NCORE):
        deg_local = deg[c * SH:(c + 1) * SH].astype(np.int64)
        slot = _bin_pack_blocks(deg_local)
        slot_of_node[c * SH:(c + 1) * SH] = slot
        pi_inv[c, slot] = np.arange(c * SH, (c + 1) * SH)

    own = np.arange(N) // SH
    s = slot_of_node
    table_row = own * TPC + (s % P) * NB + (s // P)

    cd = row // SH
    src_row = table_row[col]
    half = (src_row >= HALF).astype(np.int64)
    dst_slot = slot_of_node[row]

    cores = []
    maxcl = maxch = 0
    for c in range(NCORE):
        m = cd == c
        er = np.stack(
            [dst_slot[m], half[m], src_row[m],
             norm[m].view(np.int32).astype(np.int64)], axis=1)
        db = er[:, 0] // P
        er = er[np.lexsort((er[:, 2], er[:, 1], db))]
        db = er[:, 0] // P
        cores.append(er)
        for b in range(NB):
            mb = db == b
            nlo = int((er[mb, 1] == 0).sum())
            nhi = int((er[mb, 1] == 1).sum())
            maxcl = max(maxcl, -(-nlo // P))
            maxch = max(maxch, -(-nhi // P))
    CL, CH = max(maxcl, 1), max(maxch, 1)
    NLO, NHI = NB * CL, NB * CH

    def wrap_idx(v):
        n = len(v)
        a = np.zeros((16, n // 16), np.int16)
        a[np.arange(n) % 16, np.arange(n) // 16] = v
        return np.tile(a, (8, 1))

    plans = []
    for c in range(NCORE):
        er = cores[c]
        db = er[:, 0] // P
        arrs = {}
        for h, C in ((0, CL), (1, CH)):
            nn = NB * C * P
            idx = np.zeros(nn, np.int64)
            nrm = np.zeros(nn, np.float32)
            dp = np.zeros(nn, np.int64)
            for b in range(NB):
                mb = (db == b) & (er[:, 1] == h)
                sub = er[mb]
                n = len(sub)
                o = b * C * P
                idx[o:o + n] = sub[:, 2] - h * HALF
                nrm[o:o + n] = sub[:, 3].astype(np.int32).view(np.float32)
                dp[o:o + n] = sub[:, 0] % P
            key = "lo" if h == 0 else "hi"
            arrs["idx_" + key] = wrap_idx(idx.astype(np.int16))
            arrs["nrm_" + key] = np.ascontiguousarray(nrm.reshape(-1, P).T)
            arrs["dp_" + key] = np.ascontiguousarray(
                dp.reshape(-1, P).T.astype(np.float32))
        plans.append(arrs)

    return dict(plans=plans, pi_inv=pi_inv, CL=CL, CH=CH, NLO=NLO, NHI=NHI)


def _build_xt(x, pi_inv):
    xp = np.zeros((TR, F_IN), np.float32)
    for c in range(NCORE):
        valid = pi_inv[c] >= 0
        xp[c * TPC:(c + 1) * TPC][valid] = x[pi_inv[c][valid]]
    return np.ascontiguousarray(xp.T)  # [96, TR] slot-major (cn, b, p)


# ------------------------------------------------------------------ device
def _build_graph(CL, CH):
    NLO, NHI = NB * CL, NB * CH
    # two SWDGE queues: alternate dma_gather descriptor generation across
    # both Q7 core pairs; single_packet=False keeps ring packets <=64 descs.
    nc = bacc.Bacc("TRN2", target_bir_lowering=False, num_devices=NCORE,
                   num_swdge_queues=2 if G > 4 else 1)

    f32, i16 = dt.float32, dt.int16
    xt_all = nc.dram_tensor("xt_all", [F_IN, TR], f32, kind="ExternalInput")
    xt_own = nc.dram_tensor("xt_own", [F_IN, TPC], f32, kind="ExternalInput")
    w10 = nc.dram_tensor("w10", [F_IN, F_HID], f32, kind="ExternalInput")
    w11 = nc.dram_tensor("w11", [F_IN, F_HID], f32, kind="ExternalInput")
    w20p = nc.dram_tensor("w20p", [F_HID, FP], f32, kind="ExternalInput")
    w21p = nc.dram_tensor("w21p", [F_HID, FP], f32, kind="ExternalInput")
    b1r = nc.dram_tensor("b1r", [1, F_HID], f32, kind="ExternalInput")
    b2r = nc.dram_tensor("b2r", [1, FP], f32, kind="ExternalInput")
    onesr = nc.dram_tensor("onesr", [1, P], f32, kind="ExternalInput")
    ident = nc.dram_tensor("ident", [P, P], f32, kind="ExternalInput")
    iota = nc.dram_tensor("iota", [P, P], f32, kind="ExternalInput")
    idx_lo = nc.dram_tensor("idx_lo", [P, NLO * 8], i16, kind="ExternalInput")
    idx_hi = nc.dram_tensor("idx_hi", [P, NHI * 8], i16, kind="ExternalInput")
    nrm_lo = nc.dram_tensor("nrm_lo", [P, NLO], f32, kind="ExternalInput")
    nrm_hi = nc.dram_tensor("nrm_hi", [P, NHI], f32, kind="ExternalInput")
    dp_lo = nc.dram_tensor("dp_lo", [P, NLO], f32, kind="ExternalInput")
    dp_hi = nc.dram_tensor("dp_hi", [P, NHI], f32, kind="ExternalInput")
    out = nc.dram_tensor("out", [P, NB, F_OUT], f32, kind="ExternalOutput")

    y1_tab = nc.dram_tensor("y1_tab", [TR, FP], f32, kind="Internal")
    z_bounce = nc.dram_tensor("z_bounce", [TPC, FP], f32, kind="Internal")
    z_full = nc.dram_tensor("z_full", [TR, FP], f32, kind="Internal")

    with tile.TileContext(nc) as tc:
        with (
            tc.tile_pool(name="const", bufs=1) as cpool,
            tc.tile_pool(name="persist", bufs=1) as ppool,
            tc.tile_pool(name="hsp", bufs=2) as hsp,
            tc.tile_pool(name="psT", bufs=2, space="PSUM") as psT,
            tc.tile_pool(name="psZ", bufs=2, space="PSUM") as psZ,
        ):
            # ---- constants / persistent loads
            def load(pool, src, shape, dtype=f32, tag=None):
                t = pool.tile(shape, dtype, tag=tag)
                nc.sync.dma_start(t[:], src[:])
                return t

            w10_t = load(cpool, w10, [F_IN, F_HID], tag="w10")
            w11_t = load(cpool, w11, [F_IN, F_HID], tag="w11")
            w20_t = load(cpool, w20p, [F_HID, FP], tag="w20")
            w21_t = load(cpool, w21p, [F_HID, FP], tag="w21")
            b1_t = load(cpool, b1r, [1, F_HID], tag="b1")
            b2_t = load(cpool, b2r, [1, FP], tag="b2")
            ones_t = load(cpool, onesr, [1, P], tag="ones")
            id_t = load(cpool, ident, [P, P], tag="ident")
            io_t = load(cpool, iota, [P, P], tag="iota")
            ixlo_t = load(cpool, idx_lo, [P, NLO * 8], i16, tag="ixlo")
            ixhi_t = load(cpool, idx_hi, [P, NHI * 8], i16, tag="ixhi")
            nlo_t = load(cpool, nrm_lo, [P, NLO], tag="nlo")
            nhi_t = load(cpool, nrm_hi, [P, NHI], tag="nhi")
            dlo_t = load(cpool, dp_lo, [P, NLO], tag="dlo")
            dhi_t = load(cpool, dp_hi, [P, NHI], tag="dhi")
            xo_t = load(ppool, xt_own, [F_IN, TPC], tag="xown")

            hT = ppool.tile([F_HID, TPC], f32, tag="hT")
            z_stage = ppool.tile([P, NB, FP], f32, tag="zst")
            out_stage = ppool.tile([P, NB, F_OUT], f32, tag="ost")

            # ---- phase A: y1 = x @ W1_1 for all nodes -> y1_tab (p-major)
            with (
                tc.tile_pool(name="xa2", bufs=2) as xa,
                tc.tile_pool(name="ya2", bufs=2) as ya,
                tc.tile_pool(name="psA", bufs=4, space="PSUM") as psA,
            ):
                BPH = min(25, NB)  # blocks per xt slice (25+24)
                for cn in range(NCORE):
                    yst = ya.tile([P, NB, FP], f32, tag="yst")
                    b0 = 0
                    for hf, nblk in ((0, BPH), (1, NB - BPH)):
                        if nblk == 0:
                            continue
                        cols = nblk * P
                        xs = xa.tile([F_IN, BPH * P], f32, tag="xs")
                        nc.sync.dma_start(
                            xs[:, :cols],
                            xt_all[:, cn * TPC + b0 * P: cn * TPC + (b0 + nblk) * P])
                        for bb in range(nblk):
                            b = b0 + bb
                            ps = psA.tile([P, F_HID], f32, tag="psy")
                            nc.tensor.matmul(
                                out=ps[:], lhsT=xs[:, bb * P:(bb + 1) * P],
                                rhs=w11_t[:], start=True, stop=True)
                            if b % 2 == 0:
                                nc.vector.tensor_copy(yst[:, b, :], ps[:])
                            else:
                                nc.scalar.copy(yst[:, b, :], ps[:])
                        b0 += nblk
                    nc.sync.dma_start(
                        y1_tab[cn * TPC:(cn + 1) * TPC, :].rearrange(
                            "(p k) f -> p k f", p=P),
                        yst[:])

            # ---- spmm pass (shared for both layers)
            def spmm_pass(tab, evict, sfx):
                NGLO = -(-NLO // G)
                NGHI = -(-NHI // G)
                with (
                    tc.tile_pool(name="mlo" + sfx, bufs=3) as mlo,
                    tc.tile_pool(name="mhi" + sfx, bufs=3) as mhi,
                    tc.tile_pool(name="ohp" + sfx, bufs=3) as ohp,
                    tc.tile_pool(name="psX" + sfx, bufs=4, space="PSUM") as psX,
                ):
                    glo_tiles = [None] * NGLO
                    ghi_tiles = [None] * NGHI

                    def get_group(is_lo, g):
                        tiles = glo_tiles if is_lo else ghi_tiles
                        if tiles[g] is not None:
                            return tiles[g]
                        NT, ixt, nt, pool, tag = (
                            (NLO, ixlo_t, nlo_t, mlo, "mlo") if is_lo else
                            (NHI, ixhi_t, nhi_t, mhi, "mhi"))
                        base = tab[0:HALF, :] if is_lo else tab[HALF:TR, :]
                        ncg = min(G, NT - g * G)
                        ni = ncg * P
                        m = pool.tile([P, G, FP], f32, tag=tag)
                        if G > 4:
                            nc.gpsimd.dma_gather(
                                m[:, :ncg, :], base,
                                ixt[:, g * G * 8:(g * G + ncg) * 8],
                                ni, ni, FP, single_packet=False,
                                queue_num=(g + (0 if is_lo else 1)) % 2)
                        else:
                            nc.gpsimd.dma_gather(
                                m[:, :ncg, :], base,
                                ixt[:, g * G * 8:(g * G + ncg) * 8],
                                ni, ni, FP)
                        nc.vector.tensor_tensor(
                            out=m[:, :ncg, :],
                            in0=m[:, :ncg, :],
                            in1=nt[:, g * G:g * G + ncg].to_broadcast([P, ncg, FP]),
                            op=mybir.AluOpType.mult)
                        tiles[g] = m
                        return m

                    for b in range(NB):
                        ps = psX.tile([P, FP], f32, tag="acc")
                        # one-hot tiles for this block
                        oh_lo = ohp.tile([P, CL * P], f32, tag="ohlo")
                        nc.vector.tensor_tensor(
                            out=oh_lo[:].rearrange("p (c j) -> p c j", c=CL),
                            in0=dlo_t[:, b * CL:(b + 1) * CL].to_broadcast([P, CL, P]),
                            in1=bass.AP(io_t[:].tensor, io_t[:].offset,
                                        [io_t[:].ap[0], [0, CL], [1, P]]),
                            op=mybir.AluOpType.is_equal)
                        oh_hi = ohp.tile([P, CH * P], f32, tag="ohhi")
                        nc.vector.tensor_tensor(
                            out=oh_hi[:].rearrange("p (c j) -> p c j", c=CH),
                            in0=dhi_t[:, b * CH:(b + 1) * CH].to_broadcast([P, CH, P]),
                            in1=bass.AP(io_t[:].tensor, io_t[:].offset,
                                        [io_t[:].ap[0], [0, CH], [1, P]]),
                            op=mybir.AluOpType.is_equal)
                        for j in range(CL):
                            q = b * CL + j
                            m = get_group(True, q // G)
                            nc.tensor.matmul(
                                out=ps[:], lhsT=oh_lo[:, j * P:(j + 1) * P],
                                rhs=m[:, q % G, :], start=(j == 0), stop=False)
                        for j in range(CH):
                            q = b * CH + j
                            m = get_group(False, q // G)
                            nc.tensor.matmul(
                                out=ps[:], lhsT=oh_hi[:, j * P:(j + 1) * P],
                                rhs=m[:, q % G, :], start=False, stop=False)
                        evict(b, ps)

            # ---- layer 1 eviction: h block
            def evict_l1(b, ps):
                nc.tensor.matmul(out=ps[:], lhsT=xo_t[:, b * P:(b + 1) * P],
                                 rhs=w10_t[:], start=False, stop=False)
                nc.tensor.matmul(out=ps[:], lhsT=ones_t[:], rhs=b1_t[:],
                                 start=False, stop=True)
                hs = hsp.tile([P, F_HID], f32, tag="hs")
                nc.scalar.activation(hs[:], ps[:], mybir.ActivationFunctionType.Relu)
                pt = psT.tile([F_HID, P], f32, tag="pt")
                nc.tensor.transpose(out=pt[:], in_=hs[:], identity=id_t[:])
                nc.vector.tensor_copy(hT[:, b * P:(b + 1) * P], pt[:])
                pz = psZ.tile([P, FP], f32, tag="pz")
                nc.tensor.matmul(out=pz[:], lhsT=hT[:, b * P:(b + 1) * P],
                                 rhs=w21_t[:], start=True, stop=True)
                nc.scalar.copy(z_stage[:, b, :], pz[:])

            spmm_pass(y1_tab, evict_l1, "a")

            # ---- exchange
            nc.sync.dma_start(
                z_bounce[:].rearrange("(p k) f -> p k f", p=P), z_stage[:])
            nc.gpsimd.collective_compute(
                "AllGather", mybir.AluOpType.bypass,
                replica_groups=[list(range(NCORE))],
                ins=[z_bounce[:].opt()],
                outs=[z_full[:].opt()],
            )

            # ---- layer 2 eviction: out block
            def evict_l2(b, ps):
                nc.tensor.matmul(out=ps[:], lhsT=hT[:, b * P:(b + 1) * P],
                                 rhs=w20_t[:], start=False, stop=False)
                nc.tensor.matmul(out=ps[:], lhsT=ones_t[:], rhs=b2_t[:],
                                 start=False, stop=True)
                if b % 2 == 0:
                    nc.scalar.copy(out_stage[:, b, :], ps[:, :F_OUT])
                else:
                    nc.vector.tensor_copy(out_stage[:, b, :], ps[:, :F_OUT])

            spmm_pass(z_full, evict_l2, "b")

            nc.sync.dma_start(out[:], out_stage[:])

    nc.compile()
    return nc


_GRAPH_CACHE = {}


def kernel(x, edge_index, W1_0, W1_1, b1, W2_0, W2_1, b2):
    x = np.asarray(x, np.float32)
    W1_0 = np.asarray(W1_0, np.float32)
    W1_1 = np.asarray(W1_1, np.float32)
    b1 = np.asarray(b1, np.float32)
    W2_0 = np.asarray(W2_0, np.float32)
    W2_1 = np.asarray(W2_1, np.float32)
    b2 = np.asarray(b2, np.float32)

    plan = _build_plan(edge_index)
    CL, CH = plan["CL"], plan["CH"]

    xt = _build_xt(x, plan["pi_inv"])
    w20p = np.zeros((F_HID, FP), np.float32); w20p[:, :F_OUT] = W2_0
    w21p = np.zeros((F_HID, FP), np.float32); w21p[:, :F_OUT] = W2_1
    b2p = np.zeros((1, FP), np.float32); b2p[0, :F_OUT] = b2
    ident = np.eye(P, dtype=np.float32)
    iota = np.tile(np.arange(P, dtype=np.float32), (P, 1))
    ones = np.ones((1, P), np.float32)

    common = dict(
        xt_all=xt, w10=W1_0, w11=W1_1,
        w20p=w20p, w21p=w21p,
        b1r=b1.reshape(1, F_HID), b2r=b2p,
        onesr=ones, ident=ident, iota=iota,
    )
    in_maps = []
    for c in range(NCORE):
        pl = plan["plans"][c]
        m = dict(common)
        m["xt_own"] = np.ascontiguousarray(xt[:, c * TPC:(c + 1) * TPC])
        m["idx_lo"] = pl["idx_lo"]; m["idx_hi"] = pl["idx_hi"]
        m["nrm_lo"] = pl["nrm_lo"]; m["nrm_hi"] = pl["nrm_hi"]
        m["dp_lo"] = pl["dp_lo"]; m["dp_hi"] = pl["dp_hi"]
        in_maps.append(m)

    global G
    res = None
    last_exc = None
    for g_try in (16, 4, 2):
        G = g_try
        key = (CL, CH, g_try)
        try:
            if key not in _GRAPH_CACHE:
                _GRAPH_CACHE[key] = _build_graph(CL, CH)
            res = run_bass_kernel_spmd(
                _GRAPH_CACHE[key], in_maps, core_ids=list(range(NCORE)))
            break
        except Exception as e:  # noqa: BLE001 - retry with safer gather size
            last_exc = e
            import time as _t
            _t.sleep(10)
    if res is None:
        raise last_exc
    kernel.last_result = res

    out_full = np.zeros((N, F_OUT), np.float32)
    pi_inv = plan["pi_inv"]
    for c in range(NCORE):
        o = res.results[c]["out"].transpose(1, 0, 2).reshape(TPC, F_OUT)
        valid = pi_inv[c] >= 0
        out_full[pi_inv[c][valid]] = o[valid]
    return out_full



# revision 15
# speedup vs baseline: 1.0797x; 1.0797x over previous
"""ChebGCN (K=2, 2-layer) on 8 Trainium2 NeuronCores — V2.

Full inputs in, full output out. Internally:
  - nodes partitioned by id across 8 cores; per-core dest nodes bin-packed
    into 49 blocks x 128 slots
  - messages reduced post-weight: tx1@W1 == segsum(norm * (x@W1)[col])
  - bf16 end-to-end (rel err ~3e-3, tolerance 2e-2)
  - gather tables: 256B rows packing TWO nodes (blocks 2a,2a+1 at same slot);
    edges sorted per (dst-block, src-block-parity) so the rhs half-slice is
    compile-time; single int16 index space (25600 rows < 32768)
  - dma_gather on 4 SWDGE queues (4 Q7 core pairs in parallel, ~2.5ns/idx)
  - scatter-add via one-hot matmuls into PSUM-resident accumulators
    (2 waves of 26/23 blocks, 4-bank mega tiles)
  - layer-2 source values exchanged with one bf16 AllGather (6.4MB)
Host does sharding prep (sort/pad/index building) and output reassembly only.
"""
import sys

for _p in ("/opt/trn_rl_repo",):
    if _p not in sys.path:
        sys.path.insert(0, _p)

import numpy as np
import ml_dtypes
import concourse.bass as bass
import concourse.bacc as bacc
import concourse.mybir as mybir
import concourse.tile as tile
from concourse.bass_utils import run_bass_kernel_spmd

N = 50000
E = 800000
NCORE = 8
SH = 6250            # nodes per core
NB = 49              # dest blocks per core
P = 128
PAIRS = 25           # block pairs per core (49 blocks + 1 pad)
TROWS = PAIRS * P    # 3200 table rows per core
TR2 = NCORE * TROWS  # 25600 rows total (< 32768 -> single int16 space)
TPC = NB * P         # 6272 slots per core
F_IN, F_HID, F_OUT = 96, 64, 40
EL = 128             # bf16 elements per table row (256B)
G = 32               # chunks per gather group
NQ = 4               # SWDGE queues

dt = mybir.dt
BF = ml_dtypes.bfloat16
DEBUG_DUMPS = False


# ----------------------------------------------------------------- host prep
def _bin_pack_blocks(deg_local):
    order = np.argsort(-deg_local, kind="stable")
    loads = np.zeros(NB, np.int64)
    counts = np.zeros(NB, np.int32)
    slot = np.full(SH, -1, np.int64)
    big = np.iinfo(np.int64).max
    for l in order:
        b = int(np.argmin(np.where(counts < P, loads, big)))
        slot[l] = b * P + counts[b]
        counts[b] += 1
        loads[b] += deg_local[l]
    return slot


def _build_plan(edge_index):
    row = np.asarray(edge_index[0], np.int64)
    col = np.asarray(edge_index[1], np.int64)
    deg = np.bincount(row, minlength=N).astype(np.float32)
    dis = np.where(deg > 0, 1.0 / np.sqrt(np.maximum(deg, 1e-12)), 0.0).astype(np.float32)
    norm = (-dis[row] * dis[col]).astype(np.float32)

    slot_of_node = np.zeros(N, np.int64)
    pi_inv = np.full((NCORE, TPC), -1, np.int64)
    for c in range(NCORE):
        deg_local = deg[c * SH:(c + 1) * SH].astype(np.int64)
        slot = _bin_pack_blocks(deg_local)
        slot_of_node[c * SH:(c + 1) * SH] = slot
        pi_inv[c, slot] = np.arange(c * SH, (c + 1) * SH)

    own = np.arange(N) // SH
    s = slot_of_node
    blk = s // P
    pp = s % P
    table_row = own * TROWS + (blk // 2) * P + pp     # pair-packed row index
    half = blk % 2                                    # which 64-el half

    cd = row // SH                                    # dest core per edge
    dst_blk = slot_of_node[row] // P
    dst_p = slot_of_node[row] % P
    src_row = table_row[col]
    src_half = half[col]

    # per (core, dst block, src half) stream; chunks of 128, count padded to
    # the max across cores (SPMD shares one graph)
    cnts = np.zeros((NCORE, NB, 2), np.int64)
    streams = {}
    for c in range(NCORE):
        m = cd == c
        er = np.stack([dst_blk[m], src_half[m], src_row[m], dst_p[m],
                       norm[m].view(np.int32).astype(np.int64)], axis=1)
        er = er[np.lexsort((er[:, 2], er[:, 1], er[:, 0]))]
        bq = er[:, 0] * 2 + er[:, 1]
        bounds = np.searchsorted(bq, np.arange(NB * 2 + 1))
        for b in range(NB):
            for q in (0, 1):
                lo, hi = bounds[b * 2 + q], bounds[b * 2 + q + 1]
                cnts[c, b, q] = hi - lo
                streams[(c, b, q)] = er[lo:hi]
    cnt_chunks = -(-np.max(cnts, axis=0) // P)        # [NB, 2] global chunk counts
    cnt_chunks = np.maximum(cnt_chunks, 0).astype(np.int64)

    # global chunk schedule: block-major, then parity, then chunk#
    sched = []                                        # (block, half)
    for b in range(NB):
        for q in (0, 1):
            sched += [(b, q)] * int(cnt_chunks[b, q])
    TC = len(sched)
    last_chunk = {}
    for i, (b, q) in enumerate(sched):
        last_chunk[b] = i

    # per-core slot arrays
    def wrap_idx(v):
        n = len(v)
        a = np.zeros((16, n // 16), np.int16)
        a[np.arange(n) % 16, np.arange(n) // 16] = v
        return np.tile(a, (8, 1))

    plans = []
    for c in range(NCORE):
        idx = np.zeros(TC * P, np.int64)
        nrm = np.zeros(TC * P, np.float32)
        dpv = np.zeros(TC * P, np.float32)
        pos = np.zeros((NB, 2), np.int64)
        ofs = {}
        o = 0
        for i, (b, q) in enumerate(sched):
            if (b, q) not in ofs:
                ofs[(b, q)] = i * P
        for b in range(NB):
            for q in (0, 1):
                er = streams[(c, b, q)]
                n = len(er)
                if n == 0:
                    continue
                o = ofs[(b, q)]
                idx[o:o + n] = er[:, 2]
                dpv[o:o + n] = er[:, 3]
                nrm[o:o + n] = er[:, 4].astype(np.int32).view(np.float32)
        plans.append(dict(
            idxw=wrap_idx(idx.astype(np.int16)),
            nrmw=np.ascontiguousarray(nrm.reshape(TC, P).T.astype(BF)),
            dpw=np.ascontiguousarray(dpv.reshape(TC, P).T.astype(BF)),
        ))

    return dict(plans=plans, pi_inv=pi_inv, sched=sched, TC=TC,
                last_chunk=last_chunk)


def _build_xt(x, pi_inv):
    xp = np.zeros((NCORE * TPC, F_IN), np.float32)
    for c in range(NCORE):
        valid = pi_inv[c] >= 0
        xp[c * TPC:(c + 1) * TPC][valid] = x[pi_inv[c][valid]]
    return np.ascontiguousarray(xp.T.astype(BF))      # [96, 50176] slot-major


# ------------------------------------------------------------------ device
def _build_graph(sched, TC, last_chunk):
    nc = bacc.Bacc("TRN2", target_bir_lowering=False, num_devices=NCORE,
                   num_swdge_queues=NQ)
    f32, i16, bf16 = dt.float32, dt.int16, dt.bfloat16

    xt = nc.dram_tensor("xt", [F_IN, NCORE * TPC], bf16, kind="ExternalInput")
    xo = nc.dram_tensor("xo", [F_IN, TPC], bf16, kind="ExternalInput")
    w10 = nc.dram_tensor("w10", [F_IN, F_HID], bf16, kind="ExternalInput")
    w11 = nc.dram_tensor("w11", [F_IN, F_HID], bf16, kind="ExternalInput")
    w20p = nc.dram_tensor("w20p", [F_HID, F_HID], bf16, kind="ExternalInput")
    w21p = nc.dram_tensor("w21p", [F_HID, F_HID], bf16, kind="ExternalInput")
    b1r = nc.dram_tensor("b1r", [1, F_HID], bf16, kind="ExternalInput")
    b2p = nc.dram_tensor("b2p", [1, F_HID], bf16, kind="ExternalInput")
    onesr = nc.dram_tensor("onesr", [1, P], bf16, kind="ExternalInput")
    iota = nc.dram_tensor("iota", [P, P], bf16, kind="ExternalInput")
    ident = nc.dram_tensor("ident", [P, P], bf16, kind="ExternalInput")
    idxw = nc.dram_tensor("idxw", [P, TC * 8], i16, kind="ExternalInput")
    nrmw = nc.dram_tensor("nrmw", [P, TC], bf16, kind="ExternalInput")
    dpw = nc.dram_tensor("dpw", [P, TC], bf16, kind="ExternalInput")
    out = nc.dram_tensor("out", [P, NB, F_OUT], f32, kind="ExternalOutput")

    dump_kind = "ExternalOutput" if DEBUG_DUMPS else "Internal"
    y_tab = nc.dram_tensor("y_tab", [TR2, EL], bf16, kind=dump_kind)
    z_bounce = nc.dram_tensor("z_bounce", [TROWS, EL], bf16, kind="Internal")
    z_full = nc.dram_tensor("z_full", [TR2, EL], bf16, kind="Internal")
    if DEBUG_DUMPS:
        dbg_h = nc.dram_tensor("dbg_h", [F_HID, TPC], bf16, kind="ExternalOutput")

    NGRP = 0  # group counter for queue rotation

    with tile.TileContext(nc) as tc:
        with (
            tc.tile_pool(name="const", bufs=1) as cpool,
            tc.tile_pool(name="persist", bufs=1) as ppool,
            tc.tile_pool(name="xsp", bufs=2) as xsp,
            tc.tile_pool(name="ysp", bufs=2) as ysp,
            tc.tile_pool(name="hsp", bufs=2) as hsp,
            tc.tile_pool(name="mp", bufs=6) as mp,
            tc.tile_pool(name="ohp", bufs=3) as ohp,
            tc.tile_pool(name="psX", bufs=4, space="PSUM") as psX,
            tc.tile_pool(name="psA", bufs=2, space="PSUM") as psA,
            tc.tile_pool(name="psE", bufs=1, space="PSUM") as psE,
            tc.tile_pool(name="psZ", bufs=1, space="PSUM") as psZ,
        ):
            def load(pool, src, shape, dtype=bf16, tag=None):
                t = pool.tile(shape, dtype, tag=tag)
                nc.sync.dma_start(t[:], src[:])
                return t

            w10_t = load(cpool, w10, [F_IN, F_HID], tag="w10")
            w11_t = load(cpool, w11, [F_IN, F_HID], tag="w11")
            w20_t = load(cpool, w20p, [F_HID, F_HID], tag="w20")
            w21_t = load(cpool, w21p, [F_HID, F_HID], tag="w21")
            b1_t = load(cpool, b1r, [1, F_HID], tag="b1")
            b2_t = load(cpool, b2p, [1, F_HID], tag="b2")
            ones_t = load(cpool, onesr, [1, P], tag="ones")
            io_t = load(cpool, iota, [P, P], tag="iota")
            id_t = load(cpool, ident, [P, P], tag="ident")
            ix_t = load(cpool, idxw, [P, TC * 8], i16, tag="ix")
            nrm_t = load(cpool, nrmw, [P, TC], tag="nrm")
            dp_t = load(cpool, dpw, [P, TC], tag="dp")
            xo_t = load(ppool, xo, [F_IN, TPC], tag="xo")

            hT = ppool.tile([F_HID, TPC], bf16, tag="hT")
            z_stage = ppool.tile([P, PAIRS, EL], bf16, tag="zst")
            out_stage = ppool.tile([P, NB, F_OUT], f32, tag="ost")
            nc.vector.memset(z_stage[:, PAIRS - 1, F_HID:EL], 0.0)

            # ---- phase A: y_tab[r] = [y1(2r) | y1(2r+1)] for ALL nodes
            for cn in range(NCORE):
                xs = xsp.tile([F_IN, TPC], bf16, tag="xs")
                nc.sync.dma_start(xs[:], xt[:, cn * TPC:(cn + 1) * TPC])
                yst = ysp.tile([P, PAIRS, EL], bf16, tag="yst")
                nc.vector.memset(yst[:, PAIRS - 1, F_HID:EL], 0.0)
                for a in range(PAIRS):
                    ps = psA.tile([P, P], f32, tag="psa")
                    nc.tensor.matmul(
                        out=ps[:, 0:F_HID],
                        lhsT=xs[:, (2 * a) * P:(2 * a + 1) * P],
                        rhs=w11_t[:], start=True, stop=True)
                    if a < PAIRS - 1:
                        nc.tensor.matmul(
                            out=ps[:, F_HID:EL],
                            lhsT=xs[:, (2 * a + 1) * P:(2 * a + 2) * P],
                            rhs=w11_t[:], start=True, stop=True)
                        if a % 2 == 0:
                            nc.scalar.copy(yst[:, a, :], ps[:])
                        else:
                            nc.vector.tensor_copy(yst[:, a, :], ps[:])
                    else:
                        nc.scalar.copy(yst[:, a, 0:F_HID], ps[:, 0:F_HID])
                nc.sync.dma_start(
                    y_tab[cn * TROWS:(cn + 1) * TROWS, :].rearrange(
                        "(a p) e -> p a e", p=P),
                    yst[:])

            # ---- spmm pass: block-sequential accumulation (one open PSUM
            # accumulation group per bank), gathers prefetched in G-chunk
            # groups on rotating SWDGE queues
            def spmm_pass(tab, own_lhsT, own_rhs, own_bias, evict):
                nonlocal NGRP
                tiles = [None] * ((TC + G - 1) // G)

                def group_of(t):
                    nonlocal NGRP
                    gi = t // G
                    if tiles[gi] is None:
                        g0 = gi * G
                        ng = min(G, TC - g0)
                        m = mp.tile([P, G, EL], bf16, tag="m")
                        nc.gpsimd.dma_gather(
                            m[:, :ng, :], tab[:],
                            ix_t[:, g0 * 8:(g0 + ng) * 8],
                            ng * P, ng * P, EL, single_packet=False,
                            queue_num=NGRP % NQ)
                        nc.vector.tensor_tensor(
                            out=m[:, :ng, :], in0=m[:, :ng, :],
                            in1=nrm_t[:, g0:g0 + ng].to_broadcast([P, ng, EL]),
                            op=mybir.AluOpType.mult)
                        oh = ohp.tile([P, G, P], bf16, tag="oh")
                        nc.vector.tensor_tensor(
                            out=oh[:, :ng, :],
                            in0=dp_t[:, g0:g0 + ng].to_broadcast([P, ng, P]),
                            in1=bass.AP(io_t[:].tensor, io_t[:].offset,
                                        [io_t[:].ap[0], [0, ng], [1, P]]),
                            op=mybir.AluOpType.is_equal)
                        tiles[gi] = (m, oh)
                        NGRP += 1
                    return tiles[gi]

                acc = None
                prev_b = -1
                for t, (b, q) in enumerate(sched):
                    if b != prev_b:
                        acc = psX.tile([P, F_HID], f32, tag="acc")
                        nc.tensor.matmul(out=acc[:], lhsT=own_lhsT(b),
                                         rhs=own_rhs[:], start=True, stop=False)
                        nc.tensor.matmul(out=acc[:], lhsT=ones_t[:],
                                         rhs=own_bias[:], start=False, stop=False)
                        prev_b = b
                    m, oh = group_of(t)
                    j = t % G
                    nc.tensor.matmul(
                        out=acc[:], lhsT=oh[:, j, :],
                        rhs=m[:, j, q * F_HID:(q + 1) * F_HID],
                        start=False, stop=(last_chunk[b] == t))
                    if last_chunk[b] == t:
                        evict(b, acc[:])

            # ---- layer 1
            def evict_l1(b, accb):
                hs = hsp.tile([P, F_HID], bf16, tag="hs")
                nc.scalar.activation(hs[:], accb,
                                     mybir.ActivationFunctionType.Relu)
                pt = psE.tile([F_HID, P], bf16, tag="pt")
                nc.tensor.transpose(out=pt[:], in_=hs[:], identity=id_t[:])
                nc.vector.tensor_copy(hT[:, b * P:(b + 1) * P], pt[:])
                if b % 2 == 1 or b == NB - 1:
                    a = b // 2
                    zp = psZ.tile([P, 2, F_HID], f32, tag="zp")
                    nc.tensor.matmul(out=zp[:, 0, :],
                                     lhsT=hT[:, (2 * a) * P:(2 * a + 1) * P],
                                     rhs=w21_t[:], start=True, stop=True)
                    if b % 2 == 1:
                        nc.tensor.matmul(out=zp[:, 1, :],
                                         lhsT=hT[:, (2 * a + 1) * P:(2 * a + 2) * P],
                                         rhs=w21_t[:], start=True, stop=True)
                        nc.scalar.copy(z_stage[:, a, :],
                                       zp[:].rearrange("p t f -> p (t f)"))
                    else:
                        nc.scalar.copy(z_stage[:, a, 0:F_HID], zp[:, 0, :])

            spmm_pass(y_tab, lambda b: xo_t[:, b * P:(b + 1) * P],
                      w10_t, b1_t, evict_l1)

            # ---- exchange
            nc.sync.dma_start(
                z_bounce[:].rearrange("(a p) e -> p a e", p=P), z_stage[:])
            nc.gpsimd.collective_compute(
                "AllGather", mybir.AluOpType.bypass,
                replica_groups=[list(range(NCORE))],
                ins=[z_bounce[:].opt()],
                outs=[z_full[:].opt()],
            )

            # ---- layer 2
            def evict_l2(b, accb):
                if b % 2 == 0:
                    nc.vector.tensor_copy(out_stage[:, b, :], accb[:, 0:F_OUT])
                else:
                    nc.scalar.copy(out_stage[:, b, :], accb[:, 0:F_OUT])

            spmm_pass(z_full, lambda b: hT[:, b * P:(b + 1) * P],
                      w20_t, b2_t, evict_l2)

            nc.sync.dma_start(out[:], out_stage[:])
            if DEBUG_DUMPS:
                nc.sync.dma_start(dbg_h[:], hT[:])

    nc.compile()
    return nc


_GRAPH_CACHE = {}


def kernel(x, edge_index, W1_0, W1_1, b1, W2_0, W2_1, b2):
    x = np.asarray(x, np.float32)
    plan = _build_plan(edge_index)
    sched, TC, last_chunk = plan["sched"], plan["TC"], plan["last_chunk"]

    xt = _build_xt(x, plan["pi_inv"])
    w20p = np.zeros((F_HID, F_HID), np.float32); w20p[:, :F_OUT] = np.asarray(W2_0, np.float32)
    w21p = np.zeros((F_HID, F_HID), np.float32); w21p[:, :F_OUT] = np.asarray(W2_1, np.float32)
    b2pv = np.zeros((1, F_HID), np.float32); b2pv[0, :F_OUT] = np.asarray(b2, np.float32)
    common = dict(
        xt=xt,
        w10=np.asarray(W1_0, np.float32).astype(BF),
        w11=np.asarray(W1_1, np.float32).astype(BF),
        w20p=w20p.astype(BF), w21p=w21p.astype(BF),
        b1r=np.asarray(b1, np.float32).reshape(1, F_HID).astype(BF),
        b2p=b2pv.astype(BF),
        onesr=np.ones((1, P), BF),
        iota=np.tile(np.arange(P, dtype=np.float32), (P, 1)).astype(BF),
        ident=np.eye(P, dtype=np.float32).astype(BF),
    )
    in_maps = []
    for c in range(NCORE):
        m = dict(common)
        m["xo"] = np.ascontiguousarray(xt[:, c * TPC:(c + 1) * TPC])
        m.update(plan["plans"][c])
        in_maps.append(m)

    key = tuple(b * 2 + q for b, q in sched)
    if key not in _GRAPH_CACHE:
        _GRAPH_CACHE[key] = _build_graph(sched, TC, last_chunk)
    res = run_bass_kernel_spmd(
        _GRAPH_CACHE[key], in_maps, core_ids=list(range(NCORE)))
    kernel.last_result = res

    out_full = np.zeros((N, F_OUT), np.float32)
    pi_inv = plan["pi_inv"]
    for c in range(NCORE):
        o = res.results[c]["out"].transpose(1, 0, 2).reshape(TPC, F_OUT)
        valid = pi_inv[c] >= 0
        out_full[pi_inv[c][valid]] = o[valid]
    return out_full


# revision 17
# speedup vs baseline: 1.1821x; 1.0948x over previous
"""ChebGCN (K=2, 2-layer) on 8 Trainium2 NeuronCores — V2.

Full inputs in, full output out. Internally:
  - nodes partitioned by id across 8 cores; per-core dest nodes bin-packed
    into 49 blocks x 128 slots
  - messages reduced post-weight: tx1@W1 == segsum(norm * (x@W1)[col])
  - bf16 end-to-end (rel err ~3e-3, tolerance 2e-2)
  - gather tables: 256B rows packing TWO nodes (blocks 2a,2a+1 at same slot);
    edges sorted per (dst-block, src-block-parity) so the rhs half-slice is
    compile-time; single int16 index space (25600 rows < 32768)
  - dma_gather on 4 SWDGE queues (4 Q7 core pairs in parallel, ~2.5ns/idx)
  - scatter-add via one-hot matmuls into PSUM-resident accumulators
    (2 waves of 26/23 blocks, 4-bank mega tiles)
  - layer-2 source values exchanged with one bf16 AllGather (6.4MB)
Host does sharding prep (sort/pad/index building) and output reassembly only.
"""
import sys

for _p in ("/opt/trn_rl_repo",):
    if _p not in sys.path:
        sys.path.insert(0, _p)

import numpy as np
import ml_dtypes
import concourse.bass as bass
import concourse.bacc as bacc
import concourse.mybir as mybir
import concourse.tile as tile
from concourse.bass_utils import run_bass_kernel_spmd

N = 50000
E = 800000
NCORE = 8
SH = 6250            # nodes per core
NB = 49              # dest blocks per core
P = 128
PAIRS = 25           # block pairs per core (49 blocks + 1 pad)
TROWS = PAIRS * P    # 3200 table rows per core
TR2 = NCORE * TROWS  # 25600 rows total (< 32768 -> single int16 space)
TPC = NB * P         # 6272 slots per core
F_IN, F_HID, F_OUT = 96, 64, 40
EL = 128             # bf16 elements per table row (256B)
G = 32               # chunks per gather group
NQ = 4               # SWDGE queues

dt = mybir.dt
BF = ml_dtypes.bfloat16
DEBUG_DUMPS = False


# ----------------------------------------------------------------- host prep
def _bin_pack_blocks(deg_local):
    order = np.argsort(-deg_local, kind="stable")
    loads = np.zeros(NB, np.int64)
    counts = np.zeros(NB, np.int32)
    slot = np.full(SH, -1, np.int64)
    big = np.iinfo(np.int64).max
    for l in order:
        b = int(np.argmin(np.where(counts < P, loads, big)))
        slot[l] = b * P + counts[b]
        counts[b] += 1
        loads[b] += deg_local[l]
    return slot


def _build_plan(edge_index):
    row = np.asarray(edge_index[0], np.int64)
    col = np.asarray(edge_index[1], np.int64)
    deg = np.bincount(row, minlength=N).astype(np.float32)
    dis = np.where(deg > 0, 1.0 / np.sqrt(np.maximum(deg, 1e-12)), 0.0).astype(np.float32)
    norm = (-dis[row] * dis[col]).astype(np.float32)

    slot_of_node = np.zeros(N, np.int64)
    pi_inv = np.full((NCORE, TPC), -1, np.int64)
    for c in range(NCORE):
        deg_local = deg[c * SH:(c + 1) * SH].astype(np.int64)
        slot = _bin_pack_blocks(deg_local)
        slot_of_node[c * SH:(c + 1) * SH] = slot
        pi_inv[c, slot] = np.arange(c * SH, (c + 1) * SH)

    own = np.arange(N) // SH
    s = slot_of_node
    blk = s // P
    pp = s % P
    table_row = own * TROWS + (blk // 2) * P + pp     # pair-packed row index
    half = blk % 2                                    # which 64-el half

    cd = row // SH                                    # dest core per edge
    dst_blk = slot_of_node[row] // P
    dst_p = slot_of_node[row] % P
    src_row = table_row[col]
    src_half = half[col]

    # per (core, dst block, src half) stream; chunks of 128, count padded to
    # the max across cores (SPMD shares one graph)
    cnts = np.zeros((NCORE, NB, 2), np.int64)
    streams = {}
    for c in range(NCORE):
        m = cd == c
        er = np.stack([dst_blk[m], src_half[m], src_row[m], dst_p[m],
                       norm[m].view(np.int32).astype(np.int64)], axis=1)
        er = er[np.lexsort((er[:, 2], er[:, 1], er[:, 0]))]
        bq = er[:, 0] * 2 + er[:, 1]
        bounds = np.searchsorted(bq, np.arange(NB * 2 + 1))
        for b in range(NB):
            for q in (0, 1):
                lo, hi = bounds[b * 2 + q], bounds[b * 2 + q + 1]
                cnts[c, b, q] = hi - lo
                streams[(c, b, q)] = er[lo:hi]
    cnt_chunks = -(-np.max(cnts, axis=0) // P)        # [NB, 2] global chunk counts
    cnt_chunks = np.maximum(cnt_chunks, 0).astype(np.int64)

    # global chunk schedule: block-major, then parity, then chunk#
    sched = []                                        # (block, half)
    for b in range(NB):
        for q in (0, 1):
            sched += [(b, q)] * int(cnt_chunks[b, q])
    TC = len(sched)
    last_chunk = {}
    for i, (b, q) in enumerate(sched):
        last_chunk[b] = i

    # per-core slot arrays
    def wrap_idx(v):
        n = len(v)
        a = np.zeros((16, n // 16), np.int16)
        a[np.arange(n) % 16, np.arange(n) // 16] = v
        return np.tile(a, (8, 1))

    plans = []
    for c in range(NCORE):
        idx = np.zeros(TC * P, np.int64)
        nrm = np.zeros(TC * P, np.float32)
        dpv = np.zeros(TC * P, np.int64)
        ofs = {}
        for i, (b, q) in enumerate(sched):
            if (b, q) not in ofs:
                ofs[(b, q)] = i * P
        for b in range(NB):
            for q in (0, 1):
                er = streams[(c, b, q)]
                n = len(er)
                if n == 0:
                    continue
                o = ofs[(b, q)]
                idx[o:o + n] = er[:, 2]
                dpv[o:o + n] = er[:, 3]
                nrm[o:o + n] = er[:, 4].astype(np.int32).view(np.float32)
        # norm-folded one-hot scatter matrices, streamed from HBM on device:
        # ohw[p, t, j] = norm of the edge at slot p of chunk t if its dest
        # slot is j else 0  (acc += ohw[:,t,:]^T @ messages applies norms)
        ohw = np.zeros((P, TC, P), BF)
        s = np.arange(TC * P)
        ohw[s % P, s // P, dpv] = nrm.astype(BF)
        plans.append(dict(
            idxw=wrap_idx(idx.astype(np.int16)),
            ohw=ohw,
            nrm=nrm, dpv=dpv,   # host-side debug only
        ))

    return dict(plans=plans, pi_inv=pi_inv, sched=sched, TC=TC,
                last_chunk=last_chunk)


def _build_xt(x, pi_inv):
    xp = np.zeros((NCORE * TPC, F_IN), np.float32)
    for c in range(NCORE):
        valid = pi_inv[c] >= 0
        xp[c * TPC:(c + 1) * TPC][valid] = x[pi_inv[c][valid]]
    return np.ascontiguousarray(xp.T.astype(BF))      # [96, 50176] slot-major


# ------------------------------------------------------------------ device
def _build_graph(sched, TC, last_chunk):
    nc = bacc.Bacc("TRN2", target_bir_lowering=False, num_devices=NCORE,
                   num_swdge_queues=NQ)
    f32, i16, bf16 = dt.float32, dt.int16, dt.bfloat16

    xt = nc.dram_tensor("xt", [F_IN, NCORE * TPC], bf16, kind="ExternalInput")
    xo = nc.dram_tensor("xo", [F_IN, TPC], bf16, kind="ExternalInput")
    w10 = nc.dram_tensor("w10", [F_IN, F_HID], bf16, kind="ExternalInput")
    w11 = nc.dram_tensor("w11", [F_IN, F_HID], bf16, kind="ExternalInput")
    w20p = nc.dram_tensor("w20p", [F_HID, F_HID], bf16, kind="ExternalInput")
    w21p = nc.dram_tensor("w21p", [F_HID, F_HID], bf16, kind="ExternalInput")
    b1r = nc.dram_tensor("b1r", [1, F_HID], bf16, kind="ExternalInput")
    b2p = nc.dram_tensor("b2p", [1, F_HID], bf16, kind="ExternalInput")
    onesr = nc.dram_tensor("onesr", [1, P], bf16, kind="ExternalInput")
    ident = nc.dram_tensor("ident", [P, P], bf16, kind="ExternalInput")
    idxw = nc.dram_tensor("idxw", [P, TC * 8], i16, kind="ExternalInput")
    ohn = nc.dram_tensor("ohn", [P, TC, P], bf16, kind="ExternalInput")
    out = nc.dram_tensor("out", [P, NB, F_OUT], f32, kind="ExternalOutput")

    dump_kind = "ExternalOutput" if DEBUG_DUMPS else "Internal"
    y_tab = nc.dram_tensor("y_tab", [TR2, EL], bf16, kind=dump_kind)
    z_bounce = nc.dram_tensor("z_bounce", [TROWS, EL], bf16, kind="Internal")
    z_full = nc.dram_tensor("z_full", [TR2, EL], bf16, kind="Internal")
    if DEBUG_DUMPS:
        dbg_h = nc.dram_tensor("dbg_h", [F_HID, TPC], bf16, kind="ExternalOutput")

    NGRP = 0  # group counter for queue rotation

    with tile.TileContext(nc) as tc:
        with (
            tc.tile_pool(name="const", bufs=1) as cpool,
            tc.tile_pool(name="persist", bufs=1) as ppool,
            tc.tile_pool(name="xsp", bufs=2) as xsp,
            tc.tile_pool(name="ysp", bufs=2) as ysp,
            tc.tile_pool(name="hsp", bufs=2) as hsp,
            tc.tile_pool(name="mp", bufs=6) as mp,
            tc.tile_pool(name="ohp", bufs=4) as ohp,
            tc.tile_pool(name="psX", bufs=4, space="PSUM") as psX,
            tc.tile_pool(name="psA", bufs=2, space="PSUM") as psA,
            tc.tile_pool(name="psE", bufs=1, space="PSUM") as psE,
            tc.tile_pool(name="psZ", bufs=1, space="PSUM") as psZ,
        ):
            def load(pool, src, shape, dtype=bf16, tag=None):
                t = pool.tile(shape, dtype, tag=tag)
                nc.sync.dma_start(t[:], src[:])
                return t

            w10_t = load(cpool, w10, [F_IN, F_HID], tag="w10")
            w11_t = load(cpool, w11, [F_IN, F_HID], tag="w11")
            w20_t = load(cpool, w20p, [F_HID, F_HID], tag="w20")
            w21_t = load(cpool, w21p, [F_HID, F_HID], tag="w21")
            b1_t = load(cpool, b1r, [1, F_HID], tag="b1")
            b2_t = load(cpool, b2p, [1, F_HID], tag="b2")
            ones_t = load(cpool, onesr, [1, P], tag="ones")
            id_t = load(cpool, ident, [P, P], tag="ident")
            ix_t = load(cpool, idxw, [P, TC * 8], i16, tag="ix")
            xo_t = load(ppool, xo, [F_IN, TPC], tag="xo")

            hT = ppool.tile([F_HID, TPC], bf16, tag="hT")
            z_stage = ppool.tile([P, PAIRS, EL], bf16, tag="zst")
            out_stage = ppool.tile([P, NB, F_OUT], f32, tag="ost")
            nc.vector.memset(z_stage[:, PAIRS - 1, F_HID:EL], 0.0)

            # ---- phase A: y_tab[r] = [y1(2r) | y1(2r+1)] for ALL nodes
            for cn in range(NCORE):
                xs = xsp.tile([F_IN, TPC], bf16, tag="xs")
                nc.sync.dma_start(xs[:], xt[:, cn * TPC:(cn + 1) * TPC])
                yst = ysp.tile([P, PAIRS, EL], bf16, tag="yst")
                nc.vector.memset(yst[:, PAIRS - 1, F_HID:EL], 0.0)
                for a in range(PAIRS):
                    ps = psA.tile([P, P], f32, tag="psa")
                    nc.tensor.matmul(
                        out=ps[:, 0:F_HID],
                        lhsT=xs[:, (2 * a) * P:(2 * a + 1) * P],
                        rhs=w11_t[:], start=True, stop=True)
                    if a < PAIRS - 1:
                        nc.tensor.matmul(
                            out=ps[:, F_HID:EL],
                            lhsT=xs[:, (2 * a + 1) * P:(2 * a + 2) * P],
                            rhs=w11_t[:], start=True, stop=True)
                        if a % 2 == 0:
                            nc.scalar.copy(yst[:, a, :], ps[:])
                        else:
                            nc.vector.tensor_copy(yst[:, a, :], ps[:])
                    else:
                        nc.scalar.copy(yst[:, a, 0:F_HID], ps[:, 0:F_HID])
                nc.sync.dma_start(
                    y_tab[cn * TROWS:(cn + 1) * TROWS, :].rearrange(
                        "(a p) e -> p a e", p=P),
                    yst[:])

            # ---- spmm pass: block-sequential accumulation (one open PSUM
            # accumulation group per bank), gathers prefetched in G-chunk
            # groups on rotating SWDGE queues
            def spmm_pass(tab, own_lhsT, own_rhs, own_bias, evict):
                nonlocal NGRP
                tiles = [None] * ((TC + G - 1) // G)

                def group_of(t):
                    nonlocal NGRP
                    gi = t // G
                    if tiles[gi] is None:
                        g0 = gi * G
                        ng = min(G, TC - g0)
                        oh = ohp.tile([P, G, P], bf16, tag="oh")
                        nc.sync.dma_start(oh[:, :ng, :], ohn[:, g0:g0 + ng, :])
                        m = mp.tile([P, G, EL], bf16, tag="m")
                        nc.gpsimd.dma_gather(
                            m[:, :ng, :], tab[:],
                            ix_t[:, g0 * 8:(g0 + ng) * 8],
                            ng * P, ng * P, EL, single_packet=False,
                            queue_num=NGRP % NQ)
                        tiles[gi] = (m, oh)
                        NGRP += 1
                    return tiles[gi]

                acc = None
                prev_b = -1
                for t, (b, q) in enumerate(sched):
                    if b != prev_b:
                        acc = psX.tile([P, F_HID], f32, tag="acc")
                        nc.tensor.matmul(out=acc[:], lhsT=own_lhsT(b),
                                         rhs=own_rhs[:], start=True, stop=False)
                        nc.tensor.matmul(out=acc[:], lhsT=ones_t[:],
                                         rhs=own_bias[:], start=False, stop=False)
                        prev_b = b
                    m, oh = group_of(t)
                    j = t % G
                    nc.tensor.matmul(
                        out=acc[:], lhsT=oh[:, j, :],
                        rhs=m[:, j, q * F_HID:(q + 1) * F_HID],
                        start=False, stop=(last_chunk[b] == t))
                    if last_chunk[b] == t:
                        evict(b, acc[:])

            # ---- layer 1
            def evict_l1(b, accb):
                hs = hsp.tile([P, F_HID], bf16, tag="hs")
                nc.scalar.activation(hs[:], accb,
                                     mybir.ActivationFunctionType.Relu)
                pt = psE.tile([F_HID, P], bf16, tag="pt")
                nc.tensor.transpose(out=pt[:], in_=hs[:], identity=id_t[:])
                nc.vector.tensor_copy(hT[:, b * P:(b + 1) * P], pt[:])
                if b % 2 == 1 or b == NB - 1:
                    a = b // 2
                    zp = psZ.tile([P, 2, F_HID], f32, tag="zp")
                    nc.tensor.matmul(out=zp[:, 0, :],
                                     lhsT=hT[:, (2 * a) * P:(2 * a + 1) * P],
                                     rhs=w21_t[:], start=True, stop=True)
                    if b % 2 == 1:
                        nc.tensor.matmul(out=zp[:, 1, :],
                                         lhsT=hT[:, (2 * a + 1) * P:(2 * a + 2) * P],
                                         rhs=w21_t[:], start=True, stop=True)
                        nc.scalar.copy(z_stage[:, a, :],
                                       zp[:].rearrange("p t f -> p (t f)"))
                    else:
                        nc.scalar.copy(z_stage[:, a, 0:F_HID], zp[:, 0, :])

            spmm_pass(y_tab, lambda b: xo_t[:, b * P:(b + 1) * P],
                      w10_t, b1_t, evict_l1)

            # ---- exchange
            nc.sync.dma_start(
                z_bounce[:].rearrange("(a p) e -> p a e", p=P), z_stage[:])
            nc.gpsimd.collective_compute(
                "AllGather", mybir.AluOpType.bypass,
                replica_groups=[list(range(NCORE))],
                ins=[z_bounce[:].opt()],
                outs=[z_full[:].opt()],
            )

            # ---- layer 2
            def evict_l2(b, accb):
                if b % 2 == 0:
                    nc.vector.tensor_copy(out_stage[:, b, :], accb[:, 0:F_OUT])
                else:
                    nc.scalar.copy(out_stage[:, b, :], accb[:, 0:F_OUT])

            spmm_pass(z_full, lambda b: hT[:, b * P:(b + 1) * P],
                      w20_t, b2_t, evict_l2)

            nc.sync.dma_start(out[:], out_stage[:])
            if DEBUG_DUMPS:
                nc.sync.dma_start(dbg_h[:], hT[:])

    nc.compile()
    return nc


_GRAPH_CACHE = {}


def kernel(x, edge_index, W1_0, W1_1, b1, W2_0, W2_1, b2):
    x = np.asarray(x, np.float32)
    plan = _build_plan(edge_index)
    sched, TC, last_chunk = plan["sched"], plan["TC"], plan["last_chunk"]

    xt = _build_xt(x, plan["pi_inv"])
    w20p = np.zeros((F_HID, F_HID), np.float32); w20p[:, :F_OUT] = np.asarray(W2_0, np.float32)
    w21p = np.zeros((F_HID, F_HID), np.float32); w21p[:, :F_OUT] = np.asarray(W2_1, np.float32)
    b2pv = np.zeros((1, F_HID), np.float32); b2pv[0, :F_OUT] = np.asarray(b2, np.float32)
    common = dict(
        xt=xt,
        w10=np.asarray(W1_0, np.float32).astype(BF),
        w11=np.asarray(W1_1, np.float32).astype(BF),
        w20p=w20p.astype(BF), w21p=w21p.astype(BF),
        b1r=np.asarray(b1, np.float32).reshape(1, F_HID).astype(BF),
        b2p=b2pv.astype(BF),
        onesr=np.ones((1, P), BF),
        ident=np.eye(P, dtype=np.float32).astype(BF),
    )
    in_maps = []
    for c in range(NCORE):
        m = dict(common)
        m["xo"] = np.ascontiguousarray(xt[:, c * TPC:(c + 1) * TPC])
        m["idxw"] = plan["plans"][c]["idxw"]
        m["ohn"] = plan["plans"][c]["ohw"]
        in_maps.append(m)

    key = tuple(b * 2 + q for b, q in sched)
    if key not in _GRAPH_CACHE:
        _GRAPH_CACHE[key] = _build_graph(sched, TC, last_chunk)
    res = run_bass_kernel_spmd(
        _GRAPH_CACHE[key], in_maps, core_ids=list(range(NCORE)))
    kernel.last_result = res

    out_full = np.zeros((N, F_OUT), np.float32)
    pi_inv = plan["pi_inv"]
    for c in range(NCORE):
        o = res.results[c]["out"].transpose(1, 0, 2).reshape(TPC, F_OUT)
        valid = pi_inv[c] >= 0
        out_full[pi_inv[c][valid]] = o[valid]
    return out_full


# revision 18
# speedup vs baseline: 1.2142x; 1.0272x over previous
"""ChebGCN (K=2, 2-layer) on 8 Trainium2 NeuronCores — V2.

Full inputs in, full output out. Internally:
  - nodes partitioned by id across 8 cores; per-core dest nodes bin-packed
    into 49 blocks x 128 slots
  - messages reduced post-weight: tx1@W1 == segsum(norm * (x@W1)[col])
  - bf16 end-to-end (rel err ~3e-3, tolerance 2e-2)
  - gather tables: 256B rows packing TWO nodes (blocks 2a,2a+1 at same slot);
    edges sorted per (dst-block, src-block-parity) so the rhs half-slice is
    compile-time; single int16 index space (25600 rows < 32768)
  - dma_gather on 4 SWDGE queues (4 Q7 core pairs in parallel, ~2.5ns/idx)
  - scatter-add via one-hot matmuls into PSUM-resident accumulators
    (2 waves of 26/23 blocks, 4-bank mega tiles)
  - layer-2 source values exchanged with one bf16 AllGather (6.4MB)
Host does sharding prep (sort/pad/index building) and output reassembly only.
"""
import sys

for _p in ("/opt/trn_rl_repo",):
    if _p not in sys.path:
        sys.path.insert(0, _p)

import numpy as np
import ml_dtypes
import concourse.bass as bass
import concourse.bacc as bacc
import concourse.mybir as mybir
import concourse.tile as tile
from concourse.bass_utils import run_bass_kernel_spmd

N = 50000
E = 800000
NCORE = 8
SH = 6250            # nodes per core
NB = 49              # dest blocks per core
P = 128
PAIRS = 25           # block pairs per core (49 blocks + 1 pad)
TROWS = PAIRS * P    # 3200 table rows per core
TR2 = NCORE * TROWS  # 25600 rows total (< 32768 -> single int16 space)
TPC = NB * P         # 6272 slots per core
F_IN, F_HID, F_OUT = 96, 64, 40
EL = 128             # bf16 elements per table row (256B)
G = 16               # chunks per gather group
NQ = 4               # SWDGE queues

dt = mybir.dt
BF = ml_dtypes.bfloat16
DEBUG_DUMPS = False


# ----------------------------------------------------------------- host prep
def _bin_pack_blocks(deg_local):
    order = np.argsort(-deg_local, kind="stable")
    loads = np.zeros(NB, np.int64)
    counts = np.zeros(NB, np.int32)
    slot = np.full(SH, -1, np.int64)
    big = np.iinfo(np.int64).max
    for l in order:
        b = int(np.argmin(np.where(counts < P, loads, big)))
        slot[l] = b * P + counts[b]
        counts[b] += 1
        loads[b] += deg_local[l]
    return slot


def _build_plan(edge_index):
    row = np.asarray(edge_index[0], np.int64)
    col = np.asarray(edge_index[1], np.int64)
    deg = np.bincount(row, minlength=N).astype(np.float32)
    dis = np.where(deg > 0, 1.0 / np.sqrt(np.maximum(deg, 1e-12)), 0.0).astype(np.float32)
    norm = (-dis[row] * dis[col]).astype(np.float32)

    slot_of_node = np.zeros(N, np.int64)
    pi_inv = np.full((NCORE, TPC), -1, np.int64)
    for c in range(NCORE):
        deg_local = deg[c * SH:(c + 1) * SH].astype(np.int64)
        slot = _bin_pack_blocks(deg_local)
        slot_of_node[c * SH:(c + 1) * SH] = slot
        pi_inv[c, slot] = np.arange(c * SH, (c + 1) * SH)

    own = np.arange(N) // SH
    s = slot_of_node
    blk = s // P
    pp = s % P
    table_row = own * TROWS + (blk // 2) * P + pp     # pair-packed row index
    half = blk % 2                                    # which 64-el half

    cd = row // SH                                    # dest core per edge
    dst_blk = slot_of_node[row] // P
    dst_p = slot_of_node[row] % P
    src_row = table_row[col]
    src_half = half[col]

    # per (core, dst block, src half) stream; chunks of 128, count padded to
    # the max across cores (SPMD shares one graph)
    cnts = np.zeros((NCORE, NB, 2), np.int64)
    streams = {}
    for c in range(NCORE):
        m = cd == c
        er = np.stack([dst_blk[m], src_half[m], src_row[m], dst_p[m],
                       norm[m].view(np.int32).astype(np.int64)], axis=1)
        er = er[np.lexsort((er[:, 2], er[:, 1], er[:, 0]))]
        bq = er[:, 0] * 2 + er[:, 1]
        bounds = np.searchsorted(bq, np.arange(NB * 2 + 1))
        for b in range(NB):
            for q in (0, 1):
                lo, hi = bounds[b * 2 + q], bounds[b * 2 + q + 1]
                cnts[c, b, q] = hi - lo
                streams[(c, b, q)] = er[lo:hi]
    cnt_chunks = -(-np.max(cnts, axis=0) // P)        # [NB, 2] global chunk counts
    cnt_chunks = np.maximum(cnt_chunks, 0).astype(np.int64)

    # global chunk schedule: block-major, then parity, then chunk#
    sched = []                                        # (block, half)
    for b in range(NB):
        for q in (0, 1):
            sched += [(b, q)] * int(cnt_chunks[b, q])
    TC = len(sched)
    last_chunk = {}
    for i, (b, q) in enumerate(sched):
        last_chunk[b] = i

    # per-core slot arrays
    def wrap_idx(v):
        n = len(v)
        a = np.zeros((16, n // 16), np.int16)
        a[np.arange(n) % 16, np.arange(n) // 16] = v
        return np.tile(a, (8, 1))

    plans = []
    for c in range(NCORE):
        idx = np.zeros(TC * P, np.int64)
        nrm = np.zeros(TC * P, np.float32)
        dpv = np.zeros(TC * P, np.int64)
        ofs = {}
        for i, (b, q) in enumerate(sched):
            if (b, q) not in ofs:
                ofs[(b, q)] = i * P
        for b in range(NB):
            for q in (0, 1):
                er = streams[(c, b, q)]
                n = len(er)
                if n == 0:
                    continue
                o = ofs[(b, q)]
                idx[o:o + n] = er[:, 2]
                dpv[o:o + n] = er[:, 3]
                nrm[o:o + n] = er[:, 4].astype(np.int32).view(np.float32)
        # norm-folded one-hot scatter matrices, streamed from HBM on device:
        # ohw[p, t, j] = norm of the edge at slot p of chunk t if its dest
        # slot is j else 0  (acc += ohw[:,t,:]^T @ messages applies norms)
        ohw = np.zeros((P, TC, P), BF)
        s = np.arange(TC * P)
        ohw[s % P, s // P, dpv] = nrm.astype(BF)
        plans.append(dict(
            idxw=wrap_idx(idx.astype(np.int16)),
            ohw=ohw,
            nrm=nrm, dpv=dpv,   # host-side debug only
        ))

    return dict(plans=plans, pi_inv=pi_inv, sched=sched, TC=TC,
                last_chunk=last_chunk)


def _build_xt(x, pi_inv):
    xp = np.zeros((NCORE * TPC, F_IN), np.float32)
    for c in range(NCORE):
        valid = pi_inv[c] >= 0
        xp[c * TPC:(c + 1) * TPC][valid] = x[pi_inv[c][valid]]
    return np.ascontiguousarray(xp.T.astype(BF))      # [96, 50176] slot-major


# ------------------------------------------------------------------ device
def _build_graph(sched, TC, last_chunk):
    nc = bacc.Bacc("TRN2", target_bir_lowering=False, num_devices=NCORE,
                   num_swdge_queues=NQ)
    f32, i16, bf16 = dt.float32, dt.int16, dt.bfloat16

    xt = nc.dram_tensor("xt", [F_IN, NCORE * TPC], bf16, kind="ExternalInput")
    xo = nc.dram_tensor("xo", [F_IN, TPC], bf16, kind="ExternalInput")
    w10 = nc.dram_tensor("w10", [F_IN, F_HID], bf16, kind="ExternalInput")
    w11 = nc.dram_tensor("w11", [F_IN, F_HID], bf16, kind="ExternalInput")
    w20p = nc.dram_tensor("w20p", [F_HID, F_HID], bf16, kind="ExternalInput")
    w21p = nc.dram_tensor("w21p", [F_HID, F_HID], bf16, kind="ExternalInput")
    b1r = nc.dram_tensor("b1r", [1, F_HID], bf16, kind="ExternalInput")
    b2p = nc.dram_tensor("b2p", [1, F_HID], bf16, kind="ExternalInput")
    onesr = nc.dram_tensor("onesr", [1, P], bf16, kind="ExternalInput")
    ident = nc.dram_tensor("ident", [P, P], bf16, kind="ExternalInput")
    idxw = nc.dram_tensor("idxw", [P, TC * 8], i16, kind="ExternalInput")
    ohn = nc.dram_tensor("ohn", [P, TC, P], bf16, kind="ExternalInput")
    out = nc.dram_tensor("out", [P, NB, F_OUT], f32, kind="ExternalOutput")

    dump_kind = "ExternalOutput" if DEBUG_DUMPS else "Internal"
    y_tab = nc.dram_tensor("y_tab", [TR2, EL], bf16, kind=dump_kind)
    z_bounce = nc.dram_tensor("z_bounce", [TROWS, EL], bf16, kind="Internal")
    z_full = nc.dram_tensor("z_full", [TR2, EL], bf16, kind="Internal")
    if DEBUG_DUMPS:
        dbg_h = nc.dram_tensor("dbg_h", [F_HID, TPC], bf16, kind="ExternalOutput")

    NGRP = 0  # group counter for queue rotation

    with tile.TileContext(nc) as tc:
        with (
            tc.tile_pool(name="const", bufs=1) as cpool,
            tc.tile_pool(name="persist", bufs=1) as ppool,
            tc.tile_pool(name="xsp", bufs=2) as xsp,
            tc.tile_pool(name="ysp", bufs=2) as ysp,
            tc.tile_pool(name="hsp", bufs=2) as hsp,
            tc.tile_pool(name="mp", bufs=12) as mp,
            tc.tile_pool(name="ohp", bufs=6) as ohp,
            tc.tile_pool(name="psX", bufs=4, space="PSUM") as psX,
            tc.tile_pool(name="psA", bufs=2, space="PSUM") as psA,
            tc.tile_pool(name="psE", bufs=1, space="PSUM") as psE,
            tc.tile_pool(name="psZ", bufs=1, space="PSUM") as psZ,
        ):
            def load(pool, src, shape, dtype=bf16, tag=None):
                t = pool.tile(shape, dtype, tag=tag)
                nc.sync.dma_start(t[:], src[:])
                return t

            w10_t = load(cpool, w10, [F_IN, F_HID], tag="w10")
            w11_t = load(cpool, w11, [F_IN, F_HID], tag="w11")
            w20_t = load(cpool, w20p, [F_HID, F_HID], tag="w20")
            w21_t = load(cpool, w21p, [F_HID, F_HID], tag="w21")
            b1_t = load(cpool, b1r, [1, F_HID], tag="b1")
            b2_t = load(cpool, b2p, [1, F_HID], tag="b2")
            ones_t = load(cpool, onesr, [1, P], tag="ones")
            id_t = load(cpool, ident, [P, P], tag="ident")
            ix_t = load(cpool, idxw, [P, TC * 8], i16, tag="ix")
            xo_t = load(ppool, xo, [F_IN, TPC], tag="xo")

            hT = ppool.tile([F_HID, TPC], bf16, tag="hT")
            z_stage = ppool.tile([P, PAIRS, EL], bf16, tag="zst")
            out_stage = ppool.tile([P, NB, F_OUT], f32, tag="ost")
            nc.vector.memset(z_stage[:, PAIRS - 1, F_HID:EL], 0.0)

            # ---- phase A: y_tab[r] = [y1(2r) | y1(2r+1)] for ALL nodes
            for cn in range(NCORE):
                xs = xsp.tile([F_IN, TPC], bf16, tag="xs")
                nc.sync.dma_start(xs[:], xt[:, cn * TPC:(cn + 1) * TPC])
                yst = ysp.tile([P, PAIRS, EL], bf16, tag="yst")
                nc.vector.memset(yst[:, PAIRS - 1, F_HID:EL], 0.0)
                for t8 in range(7):              # 8 blocks per PSUM bank
                    b0 = t8 * 8
                    nblk = min(8, NB - b0)
                    ps = psA.tile([P, 8, F_HID], f32, tag="psa")
                    for i in range(nblk):
                        nc.tensor.matmul(
                            out=ps[:, i, :],
                            lhsT=xs[:, (b0 + i) * P:(b0 + i + 1) * P],
                            rhs=w11_t[:], start=True, stop=True)
                    dst = yst[:, t8 * 4:t8 * 4 + (nblk + 1) // 2, :]
                    src = ps[:, :nblk, :].rearrange("p b f -> p (b f)")
                    dst = dst.rearrange("p a e -> p (a e)")
                    if nblk % 2 == 1:
                        dst = dst[:, 0:nblk * F_HID]
                    if t8 % 2 == 0:
                        nc.scalar.copy(dst, src)
                    else:
                        nc.vector.tensor_copy(dst, src)
                nc.sync.dma_start(
                    y_tab[cn * TROWS:(cn + 1) * TROWS, :].rearrange(
                        "(a p) e -> p a e", p=P),
                    yst[:])

            # ---- spmm pass: block-sequential accumulation (one open PSUM
            # accumulation group per bank), gathers prefetched in G-chunk
            # groups on rotating SWDGE queues
            def spmm_pass(tab, own_lhsT, own_rhs, own_bias, evict):
                nonlocal NGRP
                tiles = [None] * ((TC + G - 1) // G)

                def group_of(t):
                    nonlocal NGRP
                    gi = t // G
                    if tiles[gi] is None:
                        g0 = gi * G
                        ng = min(G, TC - g0)
                        oh = ohp.tile([P, G, P], bf16, tag="oh")
                        nc.sync.dma_start(oh[:, :ng, :], ohn[:, g0:g0 + ng, :])
                        m = mp.tile([P, G, EL], bf16, tag="m")
                        nc.gpsimd.dma_gather(
                            m[:, :ng, :], tab[:],
                            ix_t[:, g0 * 8:(g0 + ng) * 8],
                            ng * P, ng * P, EL, single_packet=False,
                            queue_num=NGRP % NQ)
                        tiles[gi] = (m, oh)
                        NGRP += 1
                    return tiles[gi]

                acc = None
                prev_b = -1
                for t, (b, q) in enumerate(sched):
                    if b != prev_b:
                        acc = psX.tile([P, F_HID], f32, tag="acc")
                        nc.tensor.matmul(out=acc[:], lhsT=own_lhsT(b),
                                         rhs=own_rhs[:], start=True, stop=False)
                        nc.tensor.matmul(out=acc[:], lhsT=ones_t[:],
                                         rhs=own_bias[:], start=False, stop=False)
                        prev_b = b
                    m, oh = group_of(t)
                    j = t % G
                    nc.tensor.matmul(
                        out=acc[:], lhsT=oh[:, j, :],
                        rhs=m[:, j, q * F_HID:(q + 1) * F_HID],
                        start=False, stop=(last_chunk[b] == t))
                    if last_chunk[b] == t:
                        evict(b, acc[:])

            # ---- layer 1
            def evict_l1(b, accb):
                hs = hsp.tile([P, F_HID], bf16, tag="hs")
                nc.scalar.activation(hs[:], accb,
                                     mybir.ActivationFunctionType.Relu)
                pt = psE.tile([F_HID, P], bf16, tag="pt")
                nc.tensor.transpose(out=pt[:], in_=hs[:], identity=id_t[:])
                nc.vector.tensor_copy(hT[:, b * P:(b + 1) * P], pt[:])
                if b % 2 == 1 or b == NB - 1:
                    a = b // 2
                    zp = psZ.tile([P, 2, F_HID], f32, tag="zp")
                    nc.tensor.matmul(out=zp[:, 0, :],
                                     lhsT=hT[:, (2 * a) * P:(2 * a + 1) * P],
                                     rhs=w21_t[:], start=True, stop=True)
                    if b % 2 == 1:
                        nc.tensor.matmul(out=zp[:, 1, :],
                                         lhsT=hT[:, (2 * a + 1) * P:(2 * a + 2) * P],
                                         rhs=w21_t[:], start=True, stop=True)
                        nc.scalar.copy(z_stage[:, a, :],
                                       zp[:].rearrange("p t f -> p (t f)"))
                    else:
                        nc.scalar.copy(z_stage[:, a, 0:F_HID], zp[:, 0, :])

            spmm_pass(y_tab, lambda b: xo_t[:, b * P:(b + 1) * P],
                      w10_t, b1_t, evict_l1)

            # ---- exchange
            nc.sync.dma_start(
                z_bounce[:].rearrange("(a p) e -> p a e", p=P), z_stage[:])
            nc.gpsimd.collective_compute(
                "AllGather", mybir.AluOpType.bypass,
                replica_groups=[list(range(NCORE))],
                ins=[z_bounce[:].opt()],
                outs=[z_full[:].opt()],
            )

            # ---- layer 2
            def evict_l2(b, accb):
                if b % 2 == 0:
                    nc.vector.tensor_copy(out_stage[:, b, :], accb[:, 0:F_OUT])
                else:
                    nc.scalar.copy(out_stage[:, b, :], accb[:, 0:F_OUT])

            spmm_pass(z_full, lambda b: hT[:, b * P:(b + 1) * P],
                      w20_t, b2_t, evict_l2)

            nc.sync.dma_start(out[:], out_stage[:])
            if DEBUG_DUMPS:
                nc.sync.dma_start(dbg_h[:], hT[:])

    nc.compile()
    return nc


_GRAPH_CACHE = {}


def kernel(x, edge_index, W1_0, W1_1, b1, W2_0, W2_1, b2):
    x = np.asarray(x, np.float32)
    plan = _build_plan(edge_index)
    sched, TC, last_chunk = plan["sched"], plan["TC"], plan["last_chunk"]

    xt = _build_xt(x, plan["pi_inv"])
    w20p = np.zeros((F_HID, F_HID), np.float32); w20p[:, :F_OUT] = np.asarray(W2_0, np.float32)
    w21p = np.zeros((F_HID, F_HID), np.float32); w21p[:, :F_OUT] = np.asarray(W2_1, np.float32)
    b2pv = np.zeros((1, F_HID), np.float32); b2pv[0, :F_OUT] = np.asarray(b2, np.float32)
    common = dict(
        xt=xt,
        w10=np.asarray(W1_0, np.float32).astype(BF),
        w11=np.asarray(W1_1, np.float32).astype(BF),
        w20p=w20p.astype(BF), w21p=w21p.astype(BF),
        b1r=np.asarray(b1, np.float32).reshape(1, F_HID).astype(BF),
        b2p=b2pv.astype(BF),
        onesr=np.ones((1, P), BF),
        ident=np.eye(P, dtype=np.float32).astype(BF),
    )
    in_maps = []
    for c in range(NCORE):
        m = dict(common)
        m["xo"] = np.ascontiguousarray(xt[:, c * TPC:(c + 1) * TPC])
        m["idxw"] = plan["plans"][c]["idxw"]
        m["ohn"] = plan["plans"][c]["ohw"]
        in_maps.append(m)

    key = tuple(b * 2 + q for b, q in sched)
    if key not in _GRAPH_CACHE:
        _GRAPH_CACHE[key] = _build_graph(sched, TC, last_chunk)
    res = run_bass_kernel_spmd(
        _GRAPH_CACHE[key], in_maps, core_ids=list(range(NCORE)))
    kernel.last_result = res

    out_full = np.zeros((N, F_OUT), np.float32)
    pi_inv = plan["pi_inv"]
    for c in range(NCORE):
        o = res.results[c]["out"].transpose(1, 0, 2).reshape(TPC, F_OUT)
        valid = pi_inv[c] >= 0
        out_full[pi_inv[c][valid]] = o[valid]
    return out_full


# revision 19
# speedup vs baseline: 1.2546x; 1.0333x over previous
"""ChebGCN (K=2, 2-layer) on 8 Trainium2 NeuronCores — V2.

Full inputs in, full output out. Internally:
  - nodes partitioned by id across 8 cores; per-core dest nodes bin-packed
    into 49 blocks x 128 slots
  - messages reduced post-weight: tx1@W1 == segsum(norm * (x@W1)[col])
  - bf16 end-to-end (rel err ~3e-3, tolerance 2e-2)
  - gather tables: 256B rows packing TWO nodes (blocks 2a,2a+1 at same slot);
    edges sorted per (dst-block, src-block-parity) so the rhs half-slice is
    compile-time; single int16 index space (25600 rows < 32768)
  - dma_gather on 4 SWDGE queues (4 Q7 core pairs in parallel, ~2.5ns/idx)
  - scatter-add via one-hot matmuls into PSUM-resident accumulators
    (2 waves of 26/23 blocks, 4-bank mega tiles)
  - layer-2 source values exchanged with one bf16 AllGather (6.4MB)
Host does sharding prep (sort/pad/index building) and output reassembly only.
"""
import sys

for _p in ("/opt/trn_rl_repo",):
    if _p not in sys.path:
        sys.path.insert(0, _p)

import numpy as np
import ml_dtypes
import concourse.bass as bass
import concourse.bacc as bacc
import concourse.mybir as mybir
import concourse.tile as tile
from concourse.bass_utils import run_bass_kernel_spmd

N = 50000
E = 800000
NCORE = 8
SH = 6250            # nodes per core
NB = 49              # dest blocks per core
P = 128
PAIRS = 25           # block pairs per core (49 blocks + 1 pad)
TROWS = PAIRS * P    # 3200 table rows per core
TR2 = NCORE * TROWS  # 25600 rows total (< 32768 -> single int16 space)
TPC = NB * P         # 6272 slots per core
F_IN, F_HID, F_OUT = 96, 64, 40
EL = 128             # bf16 elements per table row (256B)
G = 16               # chunks per gather group
NQ = 4               # SWDGE queues

dt = mybir.dt
BF = ml_dtypes.bfloat16
DEBUG_DUMPS = False


# ----------------------------------------------------------------- host prep
def _bin_pack_blocks(deg_local):
    order = np.argsort(-deg_local, kind="stable")
    loads = np.zeros(NB, np.int64)
    counts = np.zeros(NB, np.int32)
    slot = np.full(SH, -1, np.int64)
    big = np.iinfo(np.int64).max
    for l in order:
        b = int(np.argmin(np.where(counts < P, loads, big)))
        slot[l] = b * P + counts[b]
        counts[b] += 1
        loads[b] += deg_local[l]
    return slot


def _build_plan(edge_index):
    row = np.asarray(edge_index[0], np.int64)
    col = np.asarray(edge_index[1], np.int64)
    deg = np.bincount(row, minlength=N).astype(np.float32)
    dis = np.where(deg > 0, 1.0 / np.sqrt(np.maximum(deg, 1e-12)), 0.0).astype(np.float32)
    norm = (-dis[row] * dis[col]).astype(np.float32)

    slot_of_node = np.zeros(N, np.int64)
    pi_inv = np.full((NCORE, TPC), -1, np.int64)
    for c in range(NCORE):
        deg_local = deg[c * SH:(c + 1) * SH].astype(np.int64)
        slot = _bin_pack_blocks(deg_local)
        slot_of_node[c * SH:(c + 1) * SH] = slot
        pi_inv[c, slot] = np.arange(c * SH, (c + 1) * SH)

    own = np.arange(N) // SH
    s = slot_of_node
    blk = s // P
    pp = s % P
    table_row = own * TROWS + pp * PAIRS + (blk // 2)  # pair-packed, p-major
    half = blk % 2                                    # which 64-el half

    cd = row // SH                                    # dest core per edge
    dst_blk = slot_of_node[row] // P
    dst_p = slot_of_node[row] % P
    src_row = table_row[col]
    src_half = half[col]

    # per (core, dst block, src half) stream; chunks of 128, count padded to
    # the max across cores (SPMD shares one graph)
    cnts = np.zeros((NCORE, NB, 2), np.int64)
    streams = {}
    for c in range(NCORE):
        m = cd == c
        er = np.stack([dst_blk[m], src_half[m], src_row[m], dst_p[m],
                       norm[m].view(np.int32).astype(np.int64)], axis=1)
        er = er[np.lexsort((er[:, 2], er[:, 1], er[:, 0]))]
        bq = er[:, 0] * 2 + er[:, 1]
        bounds = np.searchsorted(bq, np.arange(NB * 2 + 1))
        for b in range(NB):
            for q in (0, 1):
                lo, hi = bounds[b * 2 + q], bounds[b * 2 + q + 1]
                cnts[c, b, q] = hi - lo
                streams[(c, b, q)] = er[lo:hi]
    cnt_chunks = -(-np.max(cnts, axis=0) // P)        # [NB, 2] global chunk counts
    cnt_chunks = np.maximum(cnt_chunks, 0).astype(np.int64)

    # global chunk schedule: block-major, then parity, then chunk#
    sched = []                                        # (block, half)
    for b in range(NB):
        for q in (0, 1):
            sched += [(b, q)] * int(cnt_chunks[b, q])
    TC = len(sched)
    last_chunk = {}
    for i, (b, q) in enumerate(sched):
        last_chunk[b] = i

    # per-core slot arrays
    def wrap_idx(v):
        n = len(v)
        a = np.zeros((16, n // 16), np.int16)
        a[np.arange(n) % 16, np.arange(n) // 16] = v
        return np.tile(a, (8, 1))

    plans = []
    for c in range(NCORE):
        idx = np.zeros(TC * P, np.int64)
        nrm = np.zeros(TC * P, np.float32)
        dpv = np.zeros(TC * P, np.int64)
        ofs = {}
        for i, (b, q) in enumerate(sched):
            if (b, q) not in ofs:
                ofs[(b, q)] = i * P
        for b in range(NB):
            for q in (0, 1):
                er = streams[(c, b, q)]
                n = len(er)
                if n == 0:
                    continue
                o = ofs[(b, q)]
                idx[o:o + n] = er[:, 2]
                dpv[o:o + n] = er[:, 3]
                nrm[o:o + n] = er[:, 4].astype(np.int32).view(np.float32)
        # norm-folded one-hot scatter matrices, streamed from HBM on device:
        # ohw[p, t, j] = norm of the edge at slot p of chunk t if its dest
        # slot is j else 0  (acc += ohw[:,t,:]^T @ messages applies norms)
        ohw = np.zeros((P, TC, P), BF)
        s = np.arange(TC * P)
        ohw[s % P, s // P, dpv] = nrm.astype(BF)
        plans.append(dict(
            idxw=wrap_idx(idx.astype(np.int16)),
            ohw=ohw,
            nrm=nrm, dpv=dpv,   # host-side debug only
        ))

    return dict(plans=plans, pi_inv=pi_inv, sched=sched, TC=TC,
                last_chunk=last_chunk)


def _build_xt(x, pi_inv):
    xp = np.zeros((NCORE * TPC, F_IN), np.float32)
    for c in range(NCORE):
        valid = pi_inv[c] >= 0
        xp[c * TPC:(c + 1) * TPC][valid] = x[pi_inv[c][valid]]
    return np.ascontiguousarray(xp.T.astype(BF))      # [96, 50176] slot-major


# ------------------------------------------------------------------ device
def _build_graph(sched, TC, last_chunk):
    nc = bacc.Bacc("TRN2", target_bir_lowering=False, num_devices=NCORE,
                   num_swdge_queues=NQ)
    f32, i16, bf16 = dt.float32, dt.int16, dt.bfloat16

    xt = nc.dram_tensor("xt", [F_IN, NCORE * TPC], bf16, kind="ExternalInput")
    xo = nc.dram_tensor("xo", [F_IN, TPC], bf16, kind="ExternalInput")
    w10 = nc.dram_tensor("w10", [F_IN, F_HID], bf16, kind="ExternalInput")
    w11 = nc.dram_tensor("w11", [F_IN, F_HID], bf16, kind="ExternalInput")
    w20p = nc.dram_tensor("w20p", [F_HID, F_HID], bf16, kind="ExternalInput")
    w21p = nc.dram_tensor("w21p", [F_HID, F_HID], bf16, kind="ExternalInput")
    b1r = nc.dram_tensor("b1r", [1, F_HID], bf16, kind="ExternalInput")
    b2p = nc.dram_tensor("b2p", [1, F_HID], bf16, kind="ExternalInput")
    onesr = nc.dram_tensor("onesr", [1, P], bf16, kind="ExternalInput")
    ident = nc.dram_tensor("ident", [P, P], bf16, kind="ExternalInput")
    idxw = nc.dram_tensor("idxw", [P, TC * 8], i16, kind="ExternalInput")
    ohn = nc.dram_tensor("ohn", [P, TC, P], bf16, kind="ExternalInput")
    out = nc.dram_tensor("out", [P, NB, F_OUT], f32, kind="ExternalOutput")

    dump_kind = "ExternalOutput" if DEBUG_DUMPS else "Internal"
    y_tab = nc.dram_tensor("y_tab", [TR2, EL], bf16, kind=dump_kind)
    z_bounce = nc.dram_tensor("z_bounce", [TROWS, EL], bf16, kind="Internal")
    z_full = nc.dram_tensor("z_full", [TR2, EL], bf16, kind="Internal")
    if DEBUG_DUMPS:
        dbg_h = nc.dram_tensor("dbg_h", [F_HID, TPC], bf16, kind="ExternalOutput")

    NGRP = 0  # group counter for queue rotation

    with tile.TileContext(nc) as tc:
        with (
            tc.tile_pool(name="const", bufs=1) as cpool,
            tc.tile_pool(name="persist", bufs=1) as ppool,
            tc.tile_pool(name="xsp", bufs=2) as xsp,
            tc.tile_pool(name="ysp", bufs=2) as ysp,
            tc.tile_pool(name="hsp", bufs=2) as hsp,
            tc.tile_pool(name="mp", bufs=12) as mp,
            tc.tile_pool(name="ohp", bufs=6) as ohp,
            tc.tile_pool(name="psX", bufs=4, space="PSUM") as psX,
            tc.tile_pool(name="psA", bufs=2, space="PSUM") as psA,
            tc.tile_pool(name="psE", bufs=1, space="PSUM") as psE,
            tc.tile_pool(name="psZ", bufs=1, space="PSUM") as psZ,
        ):
            def load(pool, src, shape, dtype=bf16, tag=None):
                t = pool.tile(shape, dtype, tag=tag)
                nc.sync.dma_start(t[:], src[:])
                return t

            w10_t = load(cpool, w10, [F_IN, F_HID], tag="w10")
            w11_t = load(cpool, w11, [F_IN, F_HID], tag="w11")
            w20_t = load(cpool, w20p, [F_HID, F_HID], tag="w20")
            w21_t = load(cpool, w21p, [F_HID, F_HID], tag="w21")
            b1_t = load(cpool, b1r, [1, F_HID], tag="b1")
            b2_t = load(cpool, b2p, [1, F_HID], tag="b2")
            ones_t = load(cpool, onesr, [1, P], tag="ones")
            id_t = load(cpool, ident, [P, P], tag="ident")
            ix_t = load(cpool, idxw, [P, TC * 8], i16, tag="ix")
            xo_t = load(ppool, xo, [F_IN, TPC], tag="xo")

            hT = ppool.tile([F_HID, TPC], bf16, tag="hT")
            z_stage = ppool.tile([P, PAIRS, EL], bf16, tag="zst")
            out_stage = ppool.tile([P, NB, F_OUT], f32, tag="ost")
            nc.vector.memset(z_stage[:, PAIRS - 1, F_HID:EL], 0.0)

            # ---- phase A: y_tab[r] = [y1(2r) | y1(2r+1)] for ALL nodes
            for cn in range(NCORE):
                xs = xsp.tile([F_IN, TPC], bf16, tag="xs")
                nc.sync.dma_start(xs[:], xt[:, cn * TPC:(cn + 1) * TPC])
                yst = ysp.tile([P, PAIRS, EL], bf16, tag="yst")
                nc.vector.memset(yst[:, PAIRS - 1, F_HID:EL], 0.0)
                for t8 in range(7):              # 8 blocks per PSUM bank
                    b0 = t8 * 8
                    nblk = min(8, NB - b0)
                    ps = psA.tile([P, 8, F_HID], f32, tag="psa")
                    for i in range(nblk):
                        nc.tensor.matmul(
                            out=ps[:, i, :],
                            lhsT=xs[:, (b0 + i) * P:(b0 + i + 1) * P],
                            rhs=w11_t[:], start=True, stop=True)
                    dst = yst[:, t8 * 4:t8 * 4 + (nblk + 1) // 2, :]
                    src = ps[:, :nblk, :].rearrange("p b f -> p (b f)")
                    dst = dst.rearrange("p a e -> p (a e)")
                    if nblk % 2 == 1:
                        dst = dst[:, 0:nblk * F_HID]
                    if t8 % 2 == 0:
                        nc.scalar.copy(dst, src)
                    else:
                        nc.vector.tensor_copy(dst, src)
                nc.sync.dma_start(
                    y_tab[cn * TROWS:(cn + 1) * TROWS, :].rearrange(
                        "(p a) e -> p a e", p=P),
                    yst[:])

            # ---- spmm pass: block-sequential accumulation (one open PSUM
            # accumulation group per bank), gathers prefetched in G-chunk
            # groups on rotating SWDGE queues
            def spmm_pass(tab, own_lhsT, own_rhs, own_bias, evict):
                nonlocal NGRP
                tiles = [None] * ((TC + G - 1) // G)

                def group_of(t):
                    nonlocal NGRP
                    gi = t // G
                    if tiles[gi] is None:
                        g0 = gi * G
                        ng = min(G, TC - g0)
                        oh = ohp.tile([P, G, P], bf16, tag="oh")
                        nc.sync.dma_start(oh[:, :ng, :], ohn[:, g0:g0 + ng, :])
                        m = mp.tile([P, G, EL], bf16, tag="m")
                        h = (ng + 1) // 2
                        qa = 2 * (NGRP % 2)
                        nc.gpsimd.dma_gather(
                            m[:, :h, :], tab[:],
                            ix_t[:, g0 * 8:(g0 + h) * 8],
                            h * P, h * P, EL, single_packet=False,
                            queue_num=qa)
                        if ng > h:
                            nc.gpsimd.dma_gather(
                                m[:, h:ng, :], tab[:],
                                ix_t[:, (g0 + h) * 8:(g0 + ng) * 8],
                                (ng - h) * P, (ng - h) * P, EL,
                                single_packet=False, queue_num=qa + 1)
                        tiles[gi] = (m, oh)
                        NGRP += 1
                    return tiles[gi]

                acc = None
                prev_b = -1
                for t, (b, q) in enumerate(sched):
                    if b != prev_b:
                        acc = psX.tile([P, F_HID], f32, tag="acc")
                        nc.tensor.matmul(out=acc[:], lhsT=own_lhsT(b),
                                         rhs=own_rhs[:], start=True, stop=False)
                        nc.tensor.matmul(out=acc[:], lhsT=ones_t[:],
                                         rhs=own_bias[:], start=False, stop=False)
                        prev_b = b
                    m, oh = group_of(t)
                    j = t % G
                    nc.tensor.matmul(
                        out=acc[:], lhsT=oh[:, j, :],
                        rhs=m[:, j, q * F_HID:(q + 1) * F_HID],
                        start=False, stop=(last_chunk[b] == t))
                    if last_chunk[b] == t:
                        evict(b, acc[:])

            # ---- layer 1
            def evict_l1(b, accb):
                hs = hsp.tile([P, F_HID], bf16, tag="hs")
                nc.scalar.activation(hs[:], accb,
                                     mybir.ActivationFunctionType.Relu)
                pt = psE.tile([F_HID, P], bf16, tag="pt")
                nc.tensor.transpose(out=pt[:], in_=hs[:], identity=id_t[:])
                nc.vector.tensor_copy(hT[:, b * P:(b + 1) * P], pt[:])
                if b % 2 == 1 or b == NB - 1:
                    a = b // 2
                    zp = psZ.tile([P, 2, F_HID], f32, tag="zp")
                    nc.tensor.matmul(out=zp[:, 0, :],
                                     lhsT=hT[:, (2 * a) * P:(2 * a + 1) * P],
                                     rhs=w21_t[:], start=True, stop=True)
                    if b % 2 == 1:
                        nc.tensor.matmul(out=zp[:, 1, :],
                                         lhsT=hT[:, (2 * a + 1) * P:(2 * a + 2) * P],
                                         rhs=w21_t[:], start=True, stop=True)
                        nc.scalar.copy(z_stage[:, a, :],
                                       zp[:].rearrange("p t f -> p (t f)"))
                    else:
                        nc.scalar.copy(z_stage[:, a, 0:F_HID], zp[:, 0, :])

            spmm_pass(y_tab, lambda b: xo_t[:, b * P:(b + 1) * P],
                      w10_t, b1_t, evict_l1)

            # ---- exchange
            nc.sync.dma_start(
                z_bounce[:].rearrange("(p a) e -> p a e", p=P), z_stage[:])
            nc.gpsimd.collective_compute(
                "AllGather", mybir.AluOpType.bypass,
                replica_groups=[list(range(NCORE))],
                ins=[z_bounce[:].opt()],
                outs=[z_full[:].opt()],
            )

            # ---- layer 2
            def evict_l2(b, accb):
                if b % 2 == 0:
                    nc.vector.tensor_copy(out_stage[:, b, :], accb[:, 0:F_OUT])
                else:
                    nc.scalar.copy(out_stage[:, b, :], accb[:, 0:F_OUT])

            spmm_pass(z_full, lambda b: hT[:, b * P:(b + 1) * P],
                      w20_t, b2_t, evict_l2)

            nc.sync.dma_start(out[:], out_stage[:])
            if DEBUG_DUMPS:
                nc.sync.dma_start(dbg_h[:], hT[:])

    nc.compile()
    return nc


_GRAPH_CACHE = {}


def kernel(x, edge_index, W1_0, W1_1, b1, W2_0, W2_1, b2):
    x = np.asarray(x, np.float32)
    plan = _build_plan(edge_index)
    sched, TC, last_chunk = plan["sched"], plan["TC"], plan["last_chunk"]

    xt = _build_xt(x, plan["pi_inv"])
    w20p = np.zeros((F_HID, F_HID), np.float32); w20p[:, :F_OUT] = np.asarray(W2_0, np.float32)
    w21p = np.zeros((F_HID, F_HID), np.float32); w21p[:, :F_OUT] = np.asarray(W2_1, np.float32)
    b2pv = np.zeros((1, F_HID), np.float32); b2pv[0, :F_OUT] = np.asarray(b2, np.float32)
    common = dict(
        xt=xt,
        w10=np.asarray(W1_0, np.float32).astype(BF),
        w11=np.asarray(W1_1, np.float32).astype(BF),
        w20p=w20p.astype(BF), w21p=w21p.astype(BF),
        b1r=np.asarray(b1, np.float32).reshape(1, F_HID).astype(BF),
        b2p=b2pv.astype(BF),
        onesr=np.ones((1, P), BF),
        ident=np.eye(P, dtype=np.float32).astype(BF),
    )
    in_maps = []
    for c in range(NCORE):
        m = dict(common)
        m["xo"] = np.ascontiguousarray(xt[:, c * TPC:(c + 1) * TPC])
        m["idxw"] = plan["plans"][c]["idxw"]
        m["ohn"] = plan["plans"][c]["ohw"]
        in_maps.append(m)

    key = tuple(b * 2 + q for b, q in sched)
    if key not in _GRAPH_CACHE:
        _GRAPH_CACHE[key] = _build_graph(sched, TC, last_chunk)
    res = run_bass_kernel_spmd(
        _GRAPH_CACHE[key], in_maps, core_ids=list(range(NCORE)))
    kernel.last_result = res

    out_full = np.zeros((N, F_OUT), np.float32)
    pi_inv = plan["pi_inv"]
    for c in range(NCORE):
        o = res.results[c]["out"].transpose(1, 0, 2).reshape(TPC, F_OUT)
        valid = pi_inv[c] >= 0
        out_full[pi_inv[c][valid]] = o[valid]
    return out_full
